# revision 18
# baseline (speedup 1.0000x reference)
"""GQA attention block (nn_Attention_2851858284851) on 8 TRN2 NeuronCores.

Sequence-parallel sharding: core c owns query blocks {c, 15-c} (128 tokens
each) so causal work is balanced across cores. Per core:
  - project q/k/v for its 256 tokens (all heads), RMSNorm + RoPE
  - all-gather K^T and V (bf16, ~256KB/rank each) across the 8 cores
  - causal attention for its 2 query blocks over all 32 heads
  - o-projection for its 256 rows (contraction over all 4096 head-dims is
    fully local -> no output collective; host concatenates rows)

SPMD: all 8 cores execute one identical instruction stream. Per-core causal
structure is encoded in input data (mask tensors), never in loop bounds or
addresses. Compute dtype bf16 (fp32 PSUM accumulation), softmax in fp32;
RMSNorm bounds |scores| <= sqrt(D) so exp needs no max-subtraction.

Scores are computed transposed S.T[k, q] so the exp pass (ACT, PSUM->SBUF)
lands P^T directly where the PV matmul wants it; a ones-column appended to V
makes the PV matmul also produce the softmax row-sums. Phases are kept
separate (proj | attention | o-proj) so the ACT function table is stable
within each phase (Square/Sqrt vs Exp swaps cost ~1.3us each).
"""

import sys

if "/opt/trn_rl_repo" not in sys.path:
    sys.path.insert(0, "/opt/trn_rl_repo")

import numpy as np
import ml_dtypes

BF16 = ml_dtypes.bfloat16

L, HID, D, H, HKV = 2048, 2048, 128, 32, 4
EPS = 1e-6
NC_ = 8
BLK = 128
NBLK = L // BLK   # 16
TPC = 2 * BLK     # tokens per core
NI = HID // 128   # 16 contraction chunks
GQ = H // HKV     # 8 q heads per kv head
ISCALE = float(1.0 / np.sqrt(D))
INTERLEAVE = True
POOLS_V2 = True
NORM_V2 = True


def core_blocks(c):
    return (c, NBLK - 1 - c)


def tok_rows(c):
    lo, hi = core_blocks(c)
    return np.r_[lo * BLK:(lo + 1) * BLK, hi * BLK:(hi + 1) * BLK]


def gpos(l):
    """Gathered (rank-major) [rank, slot] position of logical block l."""
    if l < NBLK // 2:
        return l, 0
    return NBLK - 1 - l, 1


def gcol(l):
    r, s = gpos(l)
    return 256 * r + 128 * s


def build_masks(c):
    """Causal masks (bf16 {0,1}) for transposed scores S.T[k, q].

    mAB [128, 8*128]: lo-query halves of key chunks 0-7 (hi halves of those
    chunks are always fully valid -> never multiplied).
    mC  [128, 8*128]: hi-query columns of key chunks 8-15.
    """
    lo, hi = core_blocks(c)
    tri = (np.arange(128)[None, :] >= np.arange(128)[:, None])
    ones = np.ones((128, 128), bool)
    zeros = np.zeros((128, 128), bool)

    def blk_mask(l, b):
        if l < b:
            return ones
        if l == b:
            return tri
        return zeros

    mAB = np.concatenate([blk_mask(l, lo) for l in range(8)], axis=1)
    mC = np.concatenate([blk_mask(l, hi) for l in range(8, 16)], axis=1)
    return mAB.astype(BF16), mC.astype(BF16)


def fold_rope(cos, sin, w):
    """Fold the RMSNorm weight into the RoPE tables:
    out[:, i] = qhat[:, i]*CW[:, i] + qhat[:, rot(i)]*SW[:, i], with
    rot(i) = i+64 (sign -) for i < 64, i-64 (sign +) otherwise."""
    half = D // 2
    cw = cos * w[None, :]
    sw = np.empty_like(sin)
    sw[:, :half] = -sin[:, :half] * w[None, half:]
    sw[:, half:] = sin[:, half:] * w[None, :half]
    return cw.astype(np.float32), sw.astype(np.float32)


# ---------------------------------------------------------------------------
# device graph
# ---------------------------------------------------------------------------
_COMPILED = None


def build():
    import concourse.tile as tile
    from concourse import bacc, mybir
    from concourse.masks import make_identity
    from contextlib import ExitStack

    dt = mybir.dt
    MUL = mybir.AluOpType.mult
    nc = bacc.Bacc("TRN2", target_bir_lowering=False, debug=False)

    xT_e = nc.declare_dram_parameter("xT", [HID, TPC], dt.bfloat16, isOutput=False)
    wqT_e = nc.declare_dram_parameter("wqT", [HID, H * D], dt.bfloat16, isOutput=False)
    wkvT_e = nc.declare_dram_parameter("wkvT", [HID, 2 * HKV * D], dt.bfloat16, isOutput=False)
    woT_e = nc.declare_dram_parameter("woT", [H * D, HID], dt.bfloat16, isOutput=False)
    cwq_e = nc.declare_dram_parameter("cwq", [TPC, D], dt.float32, isOutput=False)
    swq_e = nc.declare_dram_parameter("swq", [TPC, D], dt.float32, isOutput=False)
    cwk_e = nc.declare_dram_parameter("cwk", [TPC, D], dt.float32, isOutput=False)
    swk_e = nc.declare_dram_parameter("swk", [TPC, D], dt.float32, isOutput=False)
    mAB_e = nc.declare_dram_parameter("mAB", [128, 1024], dt.bfloat16, isOutput=False)
    mC_e = nc.declare_dram_parameter("mC", [128, 1024], dt.bfloat16, isOutput=False)
    out_e = nc.declare_dram_parameter("out", [TPC, HID], dt.float32, isOutput=True)

    kT_in = nc.dram_tensor("kT_in", [HKV * D, TPC], dt.bfloat16)
    kT_out = nc.dram_tensor("kT_out", [NC_ * HKV * D, TPC], dt.bfloat16, addr_space="Shared")
    v_in = nc.dram_tensor("v_in", [TPC, HKV * D], dt.bfloat16)
    v_out = nc.dram_tensor("v_out", [NC_ * TPC, HKV * D], dt.bfloat16, addr_space="Shared")

    rg = [list(range(NC_))]

    with tile.TileContext(nc) as tc, ExitStack() as ctx:
        pers = ctx.enter_context(tc.tile_pool(name="pers", bufs=1))
        wq_pool = ctx.enter_context(tc.tile_pool(name="wq", bufs=16))
        wo_pool = ctx.enter_context(tc.tile_pool(name="wo", bufs=10))
        small = ctx.enter_context(tc.tile_pool(name="small", bufs=6))
        stat = ctx.enter_context(tc.tile_pool(name="stat", bufs=8))
        ptp = ctx.enter_context(tc.tile_pool(name="pt", bufs=6))
        pp_pv = ctx.enter_context(tc.tile_pool(name="pp_pv", bufs=(3 if POOLS_V2 else 2), space="PSUM"))

        ident = pers.tile([128, 128], dt.bfloat16, tag="ident")
        make_identity(nc, ident[:])
        eps_t = pers.tile([128, 1], dt.float32, tag="eps")
        nc.vector.memset(eps_t[:], EPS)

        xT = pers.tile([128, NI, TPC], dt.bfloat16, tag="xT")
        for i in range(NI):
            nc.sync.dma_start(xT[:, i, :], xT_e[i * 128:(i + 1) * 128, :])

        wkv = pers.tile([128, NI, 2 * HKV * D], dt.bfloat16, tag="wkv")
        for i in range(NI):
            nc.sync.dma_start(wkv[:, i, :], wkvT_e[i * 128:(i + 1) * 128, :])

        cwq = pers.tile([128, 2, D], dt.float32, tag="cwq")
        swq = pers.tile([128, 2, D], dt.float32, tag="swq")
        cwk = pers.tile([128, 2, D], dt.float32, tag="cwk")
        swk = pers.tile([128, 2, D], dt.float32, tag="swk")
        for t, e in ((cwq, cwq_e), (swq, swq_e), (cwk, cwk_e), (swk, swk_e)):
            for tb in range(2):
                nc.sync.dma_start(t[:, tb, :], e[tb * 128:(tb + 1) * 128, :])

        mAB = pers.tile([128, 2, 4, 128], dt.bfloat16, tag="mAB")
        nc.sync.dma_start(mAB[:], mAB_e[:])
        mC = pers.tile([128, 1024], dt.bfloat16, tag="mC")
        nc.sync.dma_start(mC[:], mC_e[:])

        qT = pers.tile([128, H, TPC], dt.bfloat16, tag="qT")
        kT = pers.tile([128, HKV, L], dt.bfloat16, tag="kT")  # gathered col order
        v_aug = pers.tile([128, HKV, NBLK, D + 1], dt.bfloat16, tag="vaug")
        nc.vector.memset(v_aug[:], 1.0)  # ones column survives the v DMAs
        attnT = pers.tile([128, H, 2, 128], dt.bfloat16, tag="attnT")

        ADD = mybir.AluOpType.add

        def norm_stats(sb, ssq4):
            """ssq4[:, hh] = sum_d sb[:, hh*128:(hh+1)*128]^2."""
            if NORM_V2:
                for hh in range(4):
                    sqs = small.tile([128, 128], dt.float32, tag="sq", name="sqs", bufs=2)
                    sl = sb[:, hh * 128:(hh + 1) * 128]
                    nc.vector.tensor_tensor_reduce(
                        out=sqs[:], in0=sl, in1=sl, scale=1.0, scalar=0.0,
                        op0=MUL, op1=ADD, accum_out=ssq4[:, hh:hh + 1])
            else:
                for hh in range(4):
                    sqs = small.tile([128, 128], dt.float32, tag="sq", name="sqs", bufs=2)
                    nc.scalar.activation(sqs[:], sb[:, hh * 128:(hh + 1) * 128],
                                         mybir.ActivationFunctionType.Square,
                                         accum_out=ssq4[:, hh:hh + 1])

        def rstd_of(ssq4):
            """rstd4 = 1/sqrt(ssq4/D + eps)."""
            if NORM_V2:
                std4 = stat.tile([128, 4], dt.float32, tag="std", name="std4")
                nc.scalar.activation(std4[:], ssq4[:],
                                     mybir.ActivationFunctionType.Sqrt,
                                     bias=eps_t[:], scale=1.0 / D)
                rstd4 = stat.tile([128, 4], dt.float32, tag="rstd", name="rstd4")
                nc.vector.reciprocal(rstd4[:], std4[:])
                return rstd4
            std4 = stat.tile([128, 4], dt.float32, tag="std", name="std4")
            for hh in range(4):
                nc.scalar.activation(std4[:, hh:hh + 1], ssq4[:, hh:hh + 1],
                                     mybir.ActivationFunctionType.Sqrt,
                                     bias=eps_t[:], scale=1.0 / D)
            rstd4 = stat.tile([128, 4], dt.float32, tag="rstd", name="rstd4")
            nc.vector.reciprocal(rstd4[:], std4[:])
            return rstd4

        def rope_apply(sb_slice, rstd, cw, sw, out_bf):
            """normed+roped bf16 from SBUF fp32 slice [128t, 128d]."""
            half = D // 2
            a = small.tile([128, 128], dt.float32, tag="ra", name="ra", bufs=3)
            m = small.tile([128, 128], dt.float32, tag="rm", name="rm", bufs=3)
            nc.vector.scalar_tensor_tensor(out=a[:], in0=sb_slice, scalar=rstd,
                                           in1=cw, op0=MUL, op1=MUL)
            nc.vector.scalar_tensor_tensor(out=m[:, :half], in0=sb_slice[:, half:],
                                           scalar=rstd, in1=sw[:, :half],
                                           op0=MUL, op1=MUL)
            nc.vector.scalar_tensor_tensor(out=m[:, half:], in0=sb_slice[:, :half],
                                           scalar=rstd, in1=sw[:, half:],
                                           op0=MUL, op1=MUL)
            nc.vector.tensor_add(out_bf, a[:], m[:])

        # ========== projections + attention, interleaved per kv head =======
        # ACT table holds only {Sqrt, Exp} (Square runs on DVE) so the
        # interleave does not thrash activation-table loads.
        with (tc.tile_pool(name="pp_proj", bufs=(1 if POOLS_V2 else 2), space="PSUM") as pp_proj,
              tc.tile_pool(name="pp_sc", bufs=2, space="PSUM") as pp_sc):

            def kvproj():
                for tb in range(2):
                    k_ps = pp_proj.tile([128, 512], dt.float32, tag="proj", name="k_ps")
                    for i in range(NI):
                        nc.tensor.matmul(k_ps[:], xT[:, i, tb * 128:(tb + 1) * 128],
                                         wkv[:, i, 0:512], start=(i == 0),
                                         stop=(i == NI - 1))
                    ksb = small.tile([128, 512], dt.float32, tag="psb", name="ksb", bufs=3)
                    nc.vector.tensor_copy(ksb[:], k_ps[:])
                    v_ps = pp_proj.tile([128, 512], dt.float32, tag="proj", name="v_ps")
                    for i in range(NI):
                        nc.tensor.matmul(v_ps[:], xT[:, i, tb * 128:(tb + 1) * 128],
                                         wkv[:, i, 512:1024], start=(i == 0),
                                         stop=(i == NI - 1))
                    vbf = small.tile([128, 512], dt.bfloat16, tag="vbf", name="vbf", bufs=2)
                    nc.vector.tensor_copy(vbf[:], v_ps[:])
                    nc.sync.dma_start(v_in[tb * 128:(tb + 1) * 128, :], vbf[:])
                    ssq4 = stat.tile([128, 4], dt.float32, tag="ssq", name="ssq4")
                    norm_stats(ksb[:], ssq4)
                    rstd4 = rstd_of(ssq4)
                    for h in range(HKV):
                        kbf = small.tile([128, 128], dt.bfloat16, tag="kbf", name="kbf", bufs=4)
                        rope_apply(ksb[:, h * 128:(h + 1) * 128], rstd4[:, h:h + 1],
                                   cwk[:, tb, :], swk[:, tb, :], kbf[:])
                        ktp = pp_pv.tile([128, 128], dt.bfloat16, tag="pv", name="ktp")
                        nc.tensor.transpose(ktp[:], kbf[:], ident[:])
                        kts = small.tile([128, 128], dt.bfloat16, tag="kts", name="kts", bufs=4)
                        nc.vector.tensor_copy(kts[:], ktp[:])
                        nc.sync.dma_start(
                            kT_in[h * 128:(h + 1) * 128, tb * 128:(tb + 1) * 128],
                            kts[:])

            def qproj_pair(jcp):
                wts = []
                for i in range(NI):
                    w = wq_pool.tile([128, 1024], dt.bfloat16, tag="wq", name="wq")
                    nc.sync.dma_start(w[:], wqT_e[i * 128:(i + 1) * 128,
                                                  jcp * 1024:(jcp + 1) * 1024])
                    wts.append(w)
                for sub in range(2):
                    jc = 2 * jcp + sub
                    for tb in range(2):
                        q_ps = pp_proj.tile([128, 512], dt.float32, tag="proj",
                                            name="q_ps")
                        for i in range(NI):
                            nc.tensor.matmul(q_ps[:],
                                             xT[:, i, tb * 128:(tb + 1) * 128],
                                             wts[i][:, sub * 512:(sub + 1) * 512],
                                             start=(i == 0), stop=(i == NI - 1))
                        qsb = small.tile([128, 512], dt.float32, tag="psb", name="qsb", bufs=3)
                        nc.vector.tensor_copy(qsb[:], q_ps[:])
                        ssq4 = stat.tile([128, 4], dt.float32, tag="ssq", name="ssq4")
                        norm_stats(qsb[:], ssq4)
                        rstd4 = rstd_of(ssq4)
                        for hh in range(4):
                            h = jc * 4 + hh
                            qbf = small.tile([128, 128], dt.bfloat16, tag="qbf",
                                             name="qbf")
                            rope_apply(qsb[:, hh * 128:(hh + 1) * 128],
                                       rstd4[:, hh:hh + 1],
                                       cwq[:, tb, :], swq[:, tb, :], qbf[:])
                            qtp = pp_pv.tile([128, 128], dt.bfloat16, tag="pv",
                                             name="qtp")
                            nc.tensor.transpose(qtp[:], qbf[:], ident[:])
                            nc.vector.tensor_copy(qT[:, h, tb * 128:(tb + 1) * 128],
                                                  qtp[:])

            def assemble():
                for h in range(HKV):
                    for r in range(NC_):
                        nc.sync.dma_start(
                            kT[:, h, r * 256:(r + 1) * 256],
                            kT_out[r * HKV * D + h * 128:
                                   r * HKV * D + (h + 1) * 128, :])
                for h in range(HKV):
                    for l in range(NBLK):
                        r, slot = gpos(l)
                        nc.sync.dma_start(
                            v_aug[:, h, l, 0:D],
                            v_out[r * TPC + slot * 128: r * TPC + (slot + 1) * 128,
                                  h * 128:(h + 1) * 128])

            def attention(kh):
                for sub in range(GQ):
                    h = kh * GQ + sub
                    pts = []
                    for g in range(3):
                        sc = pp_sc.tile([128, 1024], dt.float32, tag="sc", name="sc")
                        if g < 2:
                            for dc in range(4):
                                l = g * 4 + dc
                                nc.tensor.matmul(
                                    sc[:, dc * 256:(dc + 1) * 256],
                                    kT[:, kh, gcol(l):gcol(l) + 128],
                                    qT[:, h, :], start=True, stop=True)
                        else:
                            for dc in range(8):
                                l = 8 + dc
                                nc.tensor.matmul(
                                    sc[:, dc * 128:(dc + 1) * 128],
                                    kT[:, kh, gcol(l):gcol(l) + 128],
                                    qT[:, h, 128:256], start=True, stop=True)
                        pt = ptp.tile([128, 1024], dt.bfloat16, tag="pt", name="pt")
                        nc.scalar.activation(pt[:], sc[:],
                                             mybir.ActivationFunctionType.Exp,
                                             scale=ISCALE)
                        if g < 2:
                            # mask only lo-query halves (hi halves always valid)
                            lo_view = pt[:].rearrange("p (c q) -> p c q", c=4)[:, :, 0:128]
                            nc.vector.tensor_tensor(lo_view, lo_view,
                                                    mAB[:, g, :, :], MUL)
                            pts.append(pt)
                        else:
                            pt2 = ptp.tile([128, 1024], dt.bfloat16, tag="pt",
                                           name="pt2")
                            nc.vector.tensor_tensor(pt2[:], pt[:], mC[:], MUL)
                            pts.append(pt2)
                    pv = [pp_pv.tile([128, D + 1], dt.float32, tag="pv", name="pv")
                          for _ in range(2)]
                    for l in range(8):
                        g, dc = divmod(l, 4)
                        nc.tensor.matmul(pv[0][:], pts[g][:, dc * 256:dc * 256 + 128],
                                         v_aug[:, kh, l, :],
                                         start=(l == 0), stop=(l == 7))
                    for l in range(NBLK):
                        if l < 8:
                            g, dc = divmod(l, 4)
                            lhs = pts[g][:, dc * 256 + 128:dc * 256 + 256]
                        else:
                            lhs = pts[2][:, (l - 8) * 128:(l - 7) * 128]
                        nc.tensor.matmul(pv[1][:], lhs, v_aug[:, kh, l, :],
                                         start=(l == 0), stop=(l == NBLK - 1))
                    for slot in range(2):
                        r_ = stat.tile([128, 1], dt.float32, tag="recip", name="r_")
                        nc.vector.reciprocal(r_[:], pv[slot][:, D:D + 1])
                        abf = small.tile([128, 128], dt.bfloat16, tag="abf", name="abf", bufs=4)
                        nc.vector.tensor_scalar_mul(abf[:], pv[slot][:, 0:D], r_[:])
                        atp = pp_pv.tile([128, 128], dt.bfloat16, tag="pv", name="atp")
                        nc.tensor.transpose(atp[:], abf[:], ident[:])
                        nc.vector.tensor_copy(attnT[:, h, slot, :], atp[:])

            kvproj()
            nc.gpsimd.collective_compute("AllGather", mybir.AluOpType.bypass,
                                         replica_groups=rg, ins=[kT_in.ap().opt()],
                                         outs=[kT_out.ap().opt()])
            nc.gpsimd.collective_compute("AllGather", mybir.AluOpType.bypass,
                                         replica_groups=rg, ins=[v_in.ap().opt()],
                                         outs=[v_out.ap().opt()])
            if INTERLEAVE:
                qproj_pair(0)
                assemble()
                attention(0)
                for p_ in range(1, 4):
                    qproj_pair(p_)
                    attention(p_)
            else:
                for p_ in range(4):
                    qproj_pair(p_)
                assemble()
                for kh_ in range(4):
                    attention(kh_)

        # ================= o-projection =====================================
        with tc.tile_pool(name="pp_o", bufs=4, space="PSUM") as pp_o:
            for mh in range(2):
                acc = [[pp_o.tile([128, 512], dt.float32, tag="o", name="oacc")
                        for _ in range(2)] for _ in range(2)]
                for j in range(H):
                    wo_t = wo_pool.tile([128, 1024], dt.bfloat16, tag="wo", name="wo")
                    nc.sync.dma_start(wo_t[:], woT_e[j * 128:(j + 1) * 128,
                                                     mh * 1024:(mh + 1) * 1024])
                    for tb in range(2):
                        for mm in range(2):
                            nc.tensor.matmul(acc[tb][mm][:], attnT[:, j, tb, :],
                                             wo_t[:, mm * 512:(mm + 1) * 512],
                                             start=(j == 0), stop=(j == H - 1))
                for tb in range(2):
                    for mm in range(2):
                        ost = small.tile([128, 512], dt.float32, tag="ost", name="ost", bufs=2)
                        nc.vector.tensor_copy(ost[:], acc[tb][mm][:])
                        nc.sync.dma_start(
                            out_e[tb * 128:(tb + 1) * 128,
                                  mh * 1024 + mm * 512: mh * 1024 + (mm + 1) * 512],
                            ost[:])

    nc.compile()
    return nc


# ---------------------------------------------------------------------------
# host wrapper
# ---------------------------------------------------------------------------

def _prep_inputs(x, wq, wk, wv, wo, q_norm_w, k_norm_w, cos, sin):
    x2 = np.asarray(x, np.float32).reshape(L, HID)
    cos2 = np.asarray(cos, np.float32).reshape(L, D)
    sin2 = np.asarray(sin, np.float32).reshape(L, D)
    xT = np.ascontiguousarray(x2.T).astype(BF16)
    wqT = np.ascontiguousarray(np.asarray(wq, np.float32).T).astype(BF16)
    wkT = np.asarray(wk, np.float32).T
    wvT = np.asarray(wv, np.float32).T
    wkvT = np.ascontiguousarray(np.concatenate([wkT, wvT], axis=1)).astype(BF16)
    woT = np.ascontiguousarray(np.asarray(wo, np.float32).T).astype(BF16)
    cwq_f, swq_f = fold_rope(cos2, sin2, np.asarray(q_norm_w, np.float32))
    cwk_f, swk_f = fold_rope(cos2, sin2, np.asarray(k_norm_w, np.float32))

    in_maps = []
    for c in range(NC_):
        rows = tok_rows(c)
        mAB, mC = build_masks(c)
        in_maps.append({
            "xT": np.ascontiguousarray(xT[:, rows]),
            "wqT": wqT, "wkvT": wkvT, "woT": woT,
            "cwq": np.ascontiguousarray(cwq_f[rows]),
            "swq": np.ascontiguousarray(swq_f[rows]),
            "cwk": np.ascontiguousarray(cwk_f[rows]),
            "swk": np.ascontiguousarray(swk_f[rows]),
            "mAB": mAB, "mC": mC,
        })
    return in_maps


def run(inputs, trace=False, repeat=2):
    global _COMPILED
    from concourse.bass_utils import run_bass_kernel_spmd

    if _COMPILED is None:
        _COMPILED = build()
    in_maps = _prep_inputs(**inputs)
    res = None
    for _ in range(max(1, repeat)):
        res = run_bass_kernel_spmd(_COMPILED, in_maps, core_ids=list(range(NC_)),
                                   trace=trace)
    out = np.empty((L, HID), np.float32)
    for c in range(NC_):
        out[tok_rows(c)] = res.results[c]["out"]
    return out.reshape(1, L, HID), res


def kernel(x, wq, wk, wv, wo, q_norm_w, k_norm_w, cos, sin):
    out, _ = run(dict(x=x, wq=wq, wk=wk, wv=wv, wo=wo, q_norm_w=q_norm_w,
                      k_norm_w=k_norm_w, cos=cos, sin=sin), trace=False)
    return out
